# revision 10
# baseline (speedup 1.0000x reference)
"""Cross-attention kernel for Trainium2, sharded over 8 NeuronCores.

Problem (per reference):
  q = wq @ x_q + bq ; k = wk @ x_kv + bk ; v = wv @ x_kv + bv   (1x1 convs)
  per head: attn = softmax(q^T k / sqrt(hd)) ; out = attn @ v^T
  y = wo @ out + bo

Sharding: core c -> (batch b = c // 4, head n = c % 4). Each core runs one
head's full attention and produces the partial output projection
y_part = wo[:, head] @ out_head; the host sums the 4 head partials per batch.

Mathematically exact simplifications (as in the previous version):
  * bk drops (per-query logit shift cancels in softmax); bv folds into the
    host-side bias (softmax rows sum to 1); scale folds into wq/bq;
    no max-subtraction (logits ~N(0,1)); softmax denominator comes from a
    ones-column in the AV stationary; normalization deferred to the host.

This version's speed structure (target: the scalar engine's exp stream is
the only pacer, ~1.08us per [128,1024] logit tile):
  * QK^T runs in fp8 DoubleRow perf mode at 0.5 cycles/row (2x bf16) with a
    residual-pair trick that keeps bf16-class accuracy: q ~ q8 + r8 and
    k ~ k8 + s8 (each fp8 e4m3 plus its fp8 residual); one DoubleRow matmul
    with moving planes (q8;r8),(r8;q8) and stationary planes (k8;k8),(s8;s8)
    contracts all four cross terms, computing (k8+s8)^T (q8+r8) exactly.
  * Input DMAs are priority-ordered: the first 512 xkv columns and first
    1024 xq columns (plus wk/wq/bq at the head of the sync ring) land first,
    so the first exp fires ~17us instead of ~29us.
  * k/v/q projections are woven into the chunk-0 QK/exp stream (PE executes
    in order; each woven projection rides the exp-paced PSUM ring).
  * v^T is produced by a dense v projection (PSUM drained by the otherwise
    idle gpsimd engine) plus one hardware DMA transpose on the vector ring
    (which carries no input traffic, so no queueing behind the 4MB of
    activations).
  * Projection PSUM drains quantize to fp8 on the fly: DVE does the low
    halves (q8 low / r8 low, k8/s8 full-width), gpsimd the high halves.
  * The AV stationary is trimmed to 65 columns (64 v^T + ones), shortening
    every AV weight load.
"""

import numpy as np
import ml_dtypes

import concourse.bacc as bacc
import concourse.mybir as mybir
import concourse.tile as tile
from concourse.bass_utils import run_bass_kernel_spmd

F32 = mybir.dt.float32
BF16 = mybir.dt.bfloat16
FP8 = mybir.dt.float8e4
DR = mybir.MatmulPerfMode.DoubleRow
SUB = mybir.AluOpType.subtract
ADD = mybir.AluOpType.add

B, C, HGT, WID = 2, 256, 64, 64
S = HGT * WID  # 4096 pixels
NH, HD = 4, 64
NCORES = 8
P = 128
IC = 1024  # i-chunk width (2 PSUM banks)
NI = S // IC  # 4
NJ = S // P  # 32 j-blocks
SCALE = HD ** -0.5
KPRI = 512   # priority xkv columns (first k/v projection slice)
QPRI = 1024  # priority xq columns (chunk-0 q projection)
PRE = 18     # chunk-0 exps banked ahead of the first AV (covers the
             # v-transpose latency without stalling the scalar engine)


def _emit(tc):
    nc = tc.nc
    xq = nc.dram_tensor("xq", [2, P, S], BF16, kind="ExternalInput").ap()
    xkv = nc.dram_tensor("xkv", [2, P, S], BF16, kind="ExternalInput").ap()
    # wqT/wkT carry duplicated columns (w^T | w^T) so the projection writes
    # both partition halves of PSUM with identical values; the fp8 quantize
    # + residual drains are then single full-width engine ops per plane.
    wqT = nc.dram_tensor("wqT", [2, P, P], BF16, kind="ExternalInput").ap()
    wkT = nc.dram_tensor("wkT", [2, P, P], BF16, kind="ExternalInput").ap()
    wvT = nc.dram_tensor("wvT", [2, P, HD], BF16, kind="ExternalInput").ap()
    woT = nc.dram_tensor("woT", [HD, C], BF16, kind="ExternalInput").ap()
    bq = nc.dram_tensor("bq", [P, 1], F32, kind="ExternalInput").ap()
    y = nc.dram_tensor("y", [2, P, S], F32, kind="ExternalOutput").ap()
    yden = nc.dram_tensor("yden", [1, S], F32, kind="ExternalOutput").ap()

    with (
        tc.tile_pool(name="const", bufs=1) as cpool,
        tc.tile_pool(name="xp", bufs=1) as xpool,
        tc.tile_pool(name="qkv", bufs=1) as qpool,
        tc.tile_pool(name="es", bufs=22) as epool,
        tc.tile_pool(name="epi", bufs=2) as fpool,
        tc.tile_pool(name="ps", bufs=2, space="PSUM") as pp,
    ):
        # ---- critical-path weights at the head of the sync HWDGE ring ----
        wq_sb = cpool.tile([P, 2 * P], BF16)
        wk_sb = cpool.tile([P, 2 * P], BF16)
        bq_sb = cpool.tile([P, 1], F32)
        for cch in range(2):
            nc.sync.dma_start(wk_sb[:, cch * P:(cch + 1) * P], wkT[cch])
        for cch in range(2):
            nc.sync.dma_start(wq_sb[:, cch * P:(cch + 1) * P], wqT[cch])
        nc.sync.dma_start(bq_sb[:], bq)

        # ---- activations: priority slices first, then the rest in pieces
        # (piece-granular DMAs so projections wait only on their own slice)
        xq_sb = [xpool.tile([P, S], BF16, tag=f"xq{i}", name=f"xq_sb{i}")
                 for i in range(2)]
        xkv_sb = [xpool.tile([P, S], BF16, tag=f"xkv{i}", name=f"xkv_sb{i}")
                  for i in range(2)]
        ring = [nc.sync, nc.scalar]
        for hh in range(2):
            ring[hh].dma_start(xkv_sb[hh][:, 0:KPRI], xkv[hh][:, 0:KPRI])
        for hh in range(2):
            ring[hh].dma_start(xq_sb[hh][:, 0:QPRI], xq[hh][:, 0:QPRI])
        for s in range(1, S // 512):
            sl = slice(s * 512, (s + 1) * 512)
            for hh in range(2):
                ring[hh].dma_start(xkv_sb[hh][:, sl], xkv[hh][:, sl])
        for t in range(1, S // 1024):
            sl = slice(t * 1024, (t + 1) * 1024)
            for hh in range(2):
                ring[hh].dma_start(xq_sb[hh][:, sl], xq[hh][:, sl])

        # ---- non-critical weights on the gpsimd SWDGE queue ----
        wv_sb = cpool.tile([P, 2 * HD], BF16)
        for cch in range(2):
            nc.gpsimd.dma_start(wv_sb[:, cch * HD:(cch + 1) * HD], wvT[cch])
        wo_sb = cpool.tile([HD, C], BF16)
        nc.gpsimd.dma_start(wo_sb[:], woT)

        # Zero bias for exp via memset (a float bias would become a DMA'd
        # const tensor queued behind the input DMAs).
        zbias_sb = cpool.tile([P, 1], F32)
        nc.vector.memset(zbias_sb[:], 0.0)

        # PE warmup burst: dense matmuls on scratch data while the input
        # DMAs are in flight. The activity monitor promotes the PE to
        # 2.4GHz after ~3.4us of sustained streaming; this keeps the PE
        # warm up to the first projection (~11us in).
        wrm_sb = cpool.tile([P, 512], BF16)
        nc.vector.memset(wrm_sb[:], 0.0)
        for w in range(20):
            wp = pp.tile([P, 512], F32, tag="st", bufs=2, name="wp")
            nc.tensor.matmul(wp[:], wrm_sb[:, 0:P], wrm_sb[:],
                             start=True, stop=True)
        # Warmup exp so the ~2.7us activation-table load happens before the
        # first real exp.
        warm_sb = cpool.tile([P, 1], BF16)
        nc.scalar.activation(warm_sb[:], zbias_sb[:],
                             mybir.ActivationFunctionType.Exp,
                             bias=zbias_sb[:])

        # ---- fp8 operand tiles ----
        # Moving planes [p, t, i]: t0 = (q8; r8), t1 = (r8; q8)
        x8 = qpool.tile([P, 2 * S], FP8)
        x8v = x8.rearrange("p (t n) -> p t n", t=2)
        # Stationary planes [p, t, j]: t0 = (k8; k8), t1 = (s8; s8)
        w8 = qpool.tile([P, 2 * S], FP8)
        w8v = w8.rearrange("p (t n) -> p t n", t=2)
        # v (dense, pre-transpose); rows 0:64 only
        v_sb = qpool.tile([HD, S], BF16)
        # v^T blocks [j-part, (block, 128)]: cols 0:64 = v^T (transpose
        # target; the hardware transpose requires this 128-stride block
        # layout — a 65-stride dest writes garbage), col 64 = ones. The AV
        # stationary slices only cols 0:65, so cols 65:128 stay untouched.
        va_sb = qpool.tile([P, NJ * P], BF16)
        va_v = va_sb.rearrange("p (j c) -> p j c", c=P)
        nc.vector.memset(va_v[:, :, HD:HD + 1], 1.0)

        # ---- projections ----
        def k_proj(s):
            sl = slice(s * 512, (s + 1) * 512)
            kp = pp.tile([P, 512], F32, tag="st", bufs=2, name="kp")
            nc.tensor.matmul(kp[:], wk_sb[:, 0:P], xkv_sb[0][:, sl],
                             start=True, stop=False)
            nc.tensor.matmul(kp[:], wk_sb[:, P:2 * P], xkv_sb[1][:, sl],
                             start=False, stop=True)
            # k8 then s8 = fp8(k - k8); kp holds (k; k) so both planes are
            # single full-width ops
            nc.vector.tensor_copy(w8v[:, 0, sl], kp[:])
            nc.vector.tensor_tensor(w8v[:, 1, sl], kp[:], w8v[:, 0, sl], SUB)

        def q_proj(t, act_assist=False):
            sl = slice(t * 512, (t + 1) * 512)
            qp = pp.tile([P, 512], F32, tag="st", bufs=2, name="qp")
            nc.tensor.matmul(qp[:], wq_sb[:, 0:P], xq_sb[0][:, sl],
                             start=True, stop=False)
            nc.tensor.matmul(qp[:], wq_sb[:, P:2 * P], xq_sb[1][:, sl],
                             start=False, stop=True)
            # qp holds (q; q); bias added during the drains. For the two
            # prologue slices the q8 planes run on the (pre-exp-idle) scalar
            # engine so the DVE chain isn't the first-exp critical path.
            LO, HI = slice(0, HD), slice(HD, P)
            if act_assist:
                nc.scalar.activation(x8v[LO, 0, sl], qp[LO, :],
                                     mybir.ActivationFunctionType.Identity,
                                     bias=bq_sb[LO])
                nc.scalar.activation(x8v[HI, 1, sl], qp[HI, :],
                                     mybir.ActivationFunctionType.Identity,
                                     bias=bq_sb[HI])
            else:
                nc.vector.tensor_scalar_add(x8v[LO, 0, sl], qp[LO, :],
                                            bq_sb[LO])
                nc.vector.tensor_scalar_add(x8v[HI, 1, sl], qp[HI, :],
                                            bq_sb[HI])
            nc.vector.scalar_tensor_tensor(
                x8v[LO, 1, sl], qp[LO, :], bq_sb[LO], x8v[LO, 0, sl], ADD, SUB)
            nc.vector.scalar_tensor_tensor(
                x8v[HI, 0, sl], qp[HI, :], bq_sb[HI], x8v[HI, 1, sl], ADD, SUB)

        def v_proj(s):
            sl = slice(s * 512, (s + 1) * 512)
            vp = pp.tile([HD, 512], F32, tag="av", bufs=2, name="vp")
            nc.tensor.matmul(vp[:], wv_sb[:, 0:HD], xkv_sb[0][:, sl],
                             start=True, stop=False)
            nc.tensor.matmul(vp[:], wv_sb[:, HD:2 * HD], xkv_sb[1][:, sl],
                             start=False, stop=True)
            nc.vector.tensor_copy(v_sb[:, sl], vp[:])

        k_proj(0)
        q_proj(0, act_assist=True)
        q_proj(1, act_assist=True)
        v_proj(0)

        # ---- attention ----
        def qk_exp(c, j):
            st = pp.tile([P, IC], F32, tag="st", bufs=2, name="st")
            for h in range(IC // 512):
                isl = slice(c * IC + h * 512, c * IC + (h + 1) * 512)
                nc.tensor.matmul(st[:, h * 512:(h + 1) * 512],
                                 w8v[:, :, j * P:(j + 1) * P],
                                 x8v[:, :, isl],
                                 start=True, stop=True, perf_mode=DR)
            et = epool.tile([P, IC], BF16, name="et")
            nc.scalar.activation(et[:], st[:],
                                 mybir.ActivationFunctionType.Exp,
                                 bias=zbias_sb[:])
            return et

        # Softmax normalization is deferred to the host: the device ships
        # y_un = wo_col @ (exp(S^T)^T V)^T plus per-pixel denominators.
        pend = [None] * NI

        def epilogue_part2(i, final=False):
            outt = pend[i]
            for oh in range(2):
                for h in range(IC // 512):
                    yp = pp.tile([P, 512], F32, tag="av", bufs=2, name="yp")
                    nc.tensor.matmul(yp[:], wo_sb[:, oh * P:(oh + 1) * P],
                                     outt[:, h * 512:(h + 1) * 512],
                                     start=True, stop=True)
                    ys = fpool.tile([P, 512], F32, name="ys")
                    if final and (oh + h) % 2 == 1:
                        nc.scalar.activation(
                            ys[:], yp[:], mybir.ActivationFunctionType.Copy)
                    else:
                        nc.vector.tensor_copy(ys[:], yp[:])
                    eng = nc.sync if oh == 0 else nc.scalar
                    eng.dma_start(
                        y[oh][:, i * IC + h * 512:i * IC + (h + 1) * 512],
                        ys[:])

        # Chunk-0 weave: remaining projections ride the exp-paced stream.
        # v projections use the (still unallocated) av-tag PSUM ring so they
        # don't interact with the qk/exp ring; k projections share the st
        # ring at at most one per j.
        weave0 = {
            0: [lambda: v_proj(1), lambda: v_proj(2)],
            1: [lambda: v_proj(3), lambda: k_proj(1)],
            2: [lambda: v_proj(4)],
            3: [lambda: v_proj(5), lambda: k_proj(2)],
            4: [lambda: v_proj(6)],
            5: [lambda: v_proj(7), lambda: k_proj(3)],
            6: [lambda: nc.sync.dma_start_transpose(
                    out=va_v[:, :, 0:HD], in_=v_sb[:])],
            7: [lambda: k_proj(4)],
            9: [lambda: k_proj(5)],
            11: [lambda: k_proj(6)],
            13: [lambda: k_proj(7)],
            14: [lambda: q_proj(2)],
            16: [lambda: q_proj(3)],
            18: [lambda: q_proj(4)],
            20: [lambda: q_proj(5)],
            22: [lambda: q_proj(6)],
            24: [lambda: q_proj(7)],
        }

        bank = []
        for j in range(PRE):
            bank.append(qk_exp(0, j))
            for fn in weave0.get(j, []):
                fn()

        for i in range(NI):
            av = pp.tile([HD + 1, IC], F32, tag="av", bufs=2, name="av")
            for j in range(NJ):
                if i > 0 and j == 8:
                    epilogue_part2(i - 1)
                if i == 0 and j < PRE:
                    et = bank[j]
                else:
                    et = qk_exp(i, j)
                    if i == 0:
                        for fn in weave0.get(j, []):
                            fn()
                for h in range(IC // 512):
                    nc.tensor.matmul(av[:, h * 512:(h + 1) * 512],
                                     va_v[:, j, 0:HD + 1],
                                     et[:, h * 512:(h + 1) * 512],
                                     start=(j == 0), stop=(j == NJ - 1))

            outt = fpool.tile([HD, IC], BF16, name="outt")
            if i == NI - 1:
                nc.vector.tensor_copy(outt[:, 0:512], av[0:HD, 0:512])
                nc.vector.tensor_copy(outt[:, 512:IC], av[0:HD, 512:IC])
            else:
                nc.vector.tensor_copy(outt[:], av[0:HD, :])
            den = fpool.tile([1, IC], F32, name="den")
            nc.vector.tensor_copy(den[:], av[HD:HD + 1, :])
            nc.gpsimd.dma_start(yden[:, i * IC:(i + 1) * IC], den[:])
            pend[i] = outt

        epilogue_part2(NI - 1, final=True)


def build():
    nc = bacc.Bacc("TRN2", target_bir_lowering=False, debug=False,
                   enable_asserts=False)
    with tile.TileContext(nc) as tc:
        _emit(tc)
    nc.compile()
    return nc


_NC_CACHE = []


def _get_nc():
    if not _NC_CACHE:
        _NC_CACHE.append(build())
    return _NC_CACHE[0]


def make_in_maps(x_q, x_kv, wq, bq, wk, bk, wv, bv, wo, bo):
    bf = ml_dtypes.bfloat16
    in_maps = []
    bo_effs = []
    for c in range(NCORES):
        b, n = divmod(c, NH)
        hs = slice(n * HD, (n + 1) * HD)
        wq_h = wq[hs].astype(np.float64) * SCALE
        bo_eff = wo[:, hs].astype(np.float64) @ bv[hs].astype(np.float64)
        if n == 0:
            bo_eff = bo_eff + bo.astype(np.float64)
        bo_effs.append(bo_eff.astype(np.float32))
        wq_dup = np.concatenate([wq_h.T, wq_h.T], axis=1)  # [256, 128]
        wk_dup = np.concatenate([wk[hs].T, wk[hs].T], axis=1)
        bq_h = (bq[hs].astype(np.float64) * SCALE).astype(np.float32)
        in_maps.append({
            "xq": np.ascontiguousarray(
                x_q[b].reshape(C, S).reshape(2, P, S)).astype(bf),
            "xkv": np.ascontiguousarray(
                x_kv[b].reshape(C, S).reshape(2, P, S)).astype(bf),
            "wqT": np.ascontiguousarray(wq_dup.reshape(2, P, P)).astype(bf),
            "wkT": np.ascontiguousarray(wk_dup.reshape(2, P, P)).astype(bf),
            "wvT": np.ascontiguousarray(
                wv[hs].T.reshape(2, P, HD)).astype(bf),
            "woT": np.ascontiguousarray(wo[:, hs].T).astype(bf),
            "bq": np.tile(bq_h, 2).reshape(P, 1),
        })
    return in_maps, bo_effs


def assemble_output(results, bo_effs):
    # y_core is the unnormalized head partial; divide by the softmax
    # denominator and add the (host-folded) bias here.
    y = np.zeros((B, C, S), np.float32)
    for c in range(NCORES):
        b = c // NH
        den = results[c]["yden"].reshape(1, S)
        y[b] += results[c]["y"].reshape(C, S) / den \
            + bo_effs[c].reshape(C, 1)
    return y.reshape(B, C, HGT, WID)


def kernel(**inputs):
    nc = _get_nc()
    in_maps, bo_effs = make_in_maps(**inputs)
    res = run_bass_kernel_spmd(nc, in_maps, list(range(NCORES)))
    return assemble_output(res.results, bo_effs)


if __name__ == "__main__":
    nc = build()
    print("built + compiled ok")
